# revision 1
# baseline (speedup 1.0000x reference)
"""LocallyConnected2d Trainium2 kernel (8-core SPMD).

out[b,o,p,q] = sum_{i,kh,kw} x[b, i, 2p+kh, 2q+kw] * weight[0, o, i, p, q, kh*3+kw]

Shipped variant "v10":
- Shard the H' (=31) output-row dim across 8 cores (4 rows/core; core 7
  gets one duplicated padding row so the SPMD program is uniform). This
  splits the dominant traffic — the 35.4MB per-location weight — 8 ways,
  unlike batch sharding which would replicate it on every core.
- Host-side im2col + layout prep (pure data movement, no math): weight
  and windows are laid out per-core as [96 partitions = (i,k)-chunk,
  free = (group: weight-cols | window-cols)] in fp16, interleaved into a
  SINGLE DRAM tensor so each group needs exactly ONE input DMA (4 total;
  each dma_start costs ~1-2us of serialized ring time here, so DMA count
  matters more than layout).
- Per block of 4 locations: one matmul per contraction chunk,
  lhsT = windows [96, 4*8], rhs = weight [96, 4*32] -> out [32, 128]
  accumulated over the 3 chunks in PSUM; only the 4 diagonal [8, 32]
  tiles are useful (extracted host-side; the 4x moving-side waste is
  free because the kernel is DMA-bound, not PE-bound).
- fp16 (not bf16): same bytes and same PE rate, but 11 mantissa bits
  -> ~3e-4 rel err vs the fp32 reference (bf16 would be ~2.3e-3).
  Variant "v9"/"v9h" (hi+lo split, 3 matmuls) reaches ~4e-6 at ~1.6x
  the time; "v2" is exact fp32 at ~39us.
"""

import os
import numpy as np
import ml_dtypes

import concourse.bacc as bacc
import concourse.mybir as mybir
import concourse.tile as tile
from concourse.bass_utils import run_bass_kernel_spmd

# Problem shapes (hardcoded per contract).
B, CI, H, W = 8, 32, 64, 64
CO = 32
KH = KW = 3
DH = DW = 2
HO = WO = 31
N_CORES = 8
RPC = 4                 # padded H'-rows per core
L = RPC * WO            # 124 locations per core
IK = CI * KH * KW       # 288 contraction
NCHUNK = 3
CK = IK // NCHUNK       # 96 partitions per chunk
GROUPS = RPC            # one compute/DMA group per H'-row
GL = L // GROUPS        # 31 locations per group

W_COLS = L * NCHUNK * CO     # 11904
WIN_COLS = L * NCHUNK * B    # 2976
OUT_COLS = L * B             # 992

_ROWS_PADDED = [[min(4 * c + j, HO - 1) for j in range(RPC)] for c in range(N_CORES)]

_NC_CACHE = {}


V2_GOUT = 256               # psum cols per group in v2: 8 col-blocks x 32 (o)
V2_OUT_COLS = V2_GOUT * GROUPS

# v4: blocked matmuls — BLK locations share one matmul (out is a BLK x BLK
# grid of [b, o] tiles; only the diagonal is useful, extracted host-side).
# fp32r needs moving free dim >= 256 for the 1 cycle/row fast path.
GLP = 32                    # padded locs per group (31 real + 1 dup)
V4_CFG = {
    "v4r": (mybir.dt.float32r, 8, np.float32),
    "v4b": (mybir.dt.bfloat16, 4, ml_dtypes.bfloat16),
    "v4b8": (mybir.dt.bfloat16, 8, ml_dtypes.bfloat16),
}


def _build_nc_v4(repeat, variant):
    dt, BLK, _ = V4_CFG[variant]
    NBLK = GLP // BLK
    gw = NCHUNK * GLP * CO   # 3072 weight cols per group
    gwin = NCHUNK * GLP * B  # 768 win cols per group
    bout = BLK * CO          # out cols per block
    orows = B * BLK          # out rows per block
    out_cols = GROUPS * NBLK * bout

    nc = bacc.Bacc("TRN2", target_bir_lowering=False)
    wT = nc.dram_tensor("wT", [GROUPS * CK, gw], dt, kind="ExternalInput")
    winT = nc.dram_tensor("winT", [GROUPS * CK, gwin], dt, kind="ExternalInput")
    out = nc.dram_tensor("out", [orows, out_cols], mybir.dt.float32, kind="ExternalOutput")

    with tile.TileContext(nc) as tc:
        with (
            tc.tile_pool(name="wp", bufs=3) as wp,
            tc.tile_pool(name="winp", bufs=3) as winp,
            tc.tile_pool(name="pp", bufs=4, space="PSUM") as pp,
            tc.tile_pool(name="op", bufs=4) as op,
        ):
            def body():
                for g in range(GROUPS):
                    wt = wp.tile([CK, gw], dt, tag="wt", name="wt")
                    nc.sync.dma_start(wt[:], wT.ap()[g * CK:(g + 1) * CK, :])
                    wint = winp.tile([CK, gwin], dt, tag="wint", name="wint")
                    nc.sync.dma_start(wint[:], winT.ap()[g * CK:(g + 1) * CK, :])

                    for bl in range(NBLK):
                        ps = pp.tile([orows, bout], mybir.dt.float32, tag="ps", name="ps")
                        for c in range(NCHUNK):
                            nc.tensor.matmul(
                                ps[:],
                                lhsT=wint[:, c * (GLP * B) + bl * (BLK * B):
                                          c * (GLP * B) + (bl + 1) * (BLK * B)],
                                rhs=wt[:, c * (GLP * CO) + bl * bout:
                                       c * (GLP * CO) + (bl + 1) * bout],
                                start=(c == 0),
                                stop=(c == NCHUNK - 1),
                            )
                        ot = op.tile([orows, bout], mybir.dt.float32, tag="ot", name="ot")
                        nc.vector.tensor_copy(ot[:], ps[:])
                        nc.sync.dma_start(
                            out.ap()[:, (g * NBLK + bl) * bout:(g * NBLK + bl + 1) * bout],
                            ot[:],
                        )

            if repeat == 1:
                body()
            else:
                with tc.For_i(0, repeat, 1):
                    body()
    nc.compile()
    return nc


def _build_nc_v5(repeat=1):
    """fp32 exact; all DMAs 128-partition; contraction 128+128+32 with the
    32-row remainder of all 4 groups packed into one 128-row tile."""
    gw = GL * CO     # 992 weight cols per (group, chunk)
    gwin = GL * B    # 248 win cols per (group, chunk)
    nc = bacc.Bacc("TRN2", target_bir_lowering=False)
    w01 = nc.dram_tensor("w01", [GROUPS * 2 * 128, gw], mybir.dt.float32, kind="ExternalInput")
    win01 = nc.dram_tensor("win01", [GROUPS * 2 * 128, gwin], mybir.dt.float32, kind="ExternalInput")
    w2 = nc.dram_tensor("w2", [GROUPS * 32, gw], mybir.dt.float32, kind="ExternalInput")
    win2 = nc.dram_tensor("win2", [GROUPS * 32, gwin], mybir.dt.float32, kind="ExternalInput")
    out = nc.dram_tensor("out", [GROUPS * 128, V2_GOUT], mybir.dt.float32, kind="ExternalOutput")

    with tile.TileContext(nc) as tc:
        with (
            tc.tile_pool(name="wp", bufs=3) as wp,
            tc.tile_pool(name="winp", bufs=3) as winp,
            tc.tile_pool(name="pp", bufs=2, space="PSUM") as pp,
            tc.tile_pool(name="op", bufs=2) as op,
        ):
            def body():
                for g in range(GROUPS):
                    wts, wints = [], []
                    for cc in range(2):
                        wt = wp.tile([128, gw], mybir.dt.float32, tag=f"wt{cc}", name=f"wt{cc}")
                        nc.sync.dma_start(
                            wt[:], w01.ap()[(g * 2 + cc) * 128:(g * 2 + cc + 1) * 128, :])
                        wint = winp.tile([128, gwin], mybir.dt.float32, tag=f"wint{cc}", name=f"wint{cc}")
                        nc.sync.dma_start(
                            wint[:], win01.ap()[(g * 2 + cc) * 128:(g * 2 + cc + 1) * 128, :])
                        wts.append(wt)
                        wints.append(wint)
                    w2t = wp.tile([32, gw], mybir.dt.float32, tag="w2t", name="w2t")
                    nc.sync.dma_start(w2t[:], w2.ap()[g * 32:(g + 1) * 32, :])
                    win2t = winp.tile([32, gwin], mybir.dt.float32, tag="win2t", name="win2t")
                    nc.sync.dma_start(win2t[:], win2.ap()[g * 32:(g + 1) * 32, :])

                    pss = [
                        pp.tile([128, V2_GOUT], mybir.dt.float32,
                                tag=f"ps{j}", name=f"ps{j}", bufs=2)
                        for j in range(4)
                    ]
                    for l in range(GL):
                        j = l % 4
                        blk = l // 4
                        dst = pss[j][32 * j:32 * j + B, blk * CO:(blk + 1) * CO]
                        for cc in range(2):
                            nc.tensor.matmul(
                                dst,
                                lhsT=wints[cc][:, l * B:(l + 1) * B],
                                rhs=wts[cc][:, l * CO:(l + 1) * CO],
                                start=(cc == 0),
                                stop=False,
                                tile_position=(0, 32 * j),
                            )
                        nc.tensor.matmul(
                            dst,
                            lhsT=win2t[:, l * B:(l + 1) * B],
                            rhs=w2t[:, l * CO:(l + 1) * CO],
                            start=False,
                            stop=True,
                            tile_position=(0, 32 * j),
                        )

                    ot = op.tile([128, V2_GOUT], mybir.dt.float32, tag="ot", name="ot")
                    for j in range(4):
                        nc.vector.tensor_copy(
                            ot[32 * j:32 * (j + 1), :],
                            pss[j][32 * j:32 * (j + 1), :],
                        )
                    nc.sync.dma_start(out.ap()[g * 128:(g + 1) * 128, :], ot[:])

            if repeat == 1:
                body()
            else:
                with tc.For_i(0, repeat, 1):
                    body()
    nc.compile()
    return nc


def _host_prep_v5(x, weight):
    x = np.ascontiguousarray(np.asarray(x, dtype=np.float32))
    weight = np.ascontiguousarray(np.asarray(weight, dtype=np.float32))
    wins = np.stack(
        [x[:, :, kh:kh + DH * HO:DH, kw:kw + DW * WO:DW]
         for kh in range(KH) for kw in range(KW)],
        axis=-1,
    )
    W2 = weight[0].transpose(1, 4, 2, 3, 0).reshape(IK, HO, WO, CO)
    W3 = wins.transpose(1, 4, 2, 3, 0).reshape(IK, HO, WO, B)
    in_maps = []
    for c in range(N_CORES):
        rows = _ROWS_PADDED[c]
        wsel = W2[:, rows]       # (288, 4, 31, CO)
        winsel = W3[:, rows]     # (288, 4, 31, B)
        # w01 rows: (g, c01, 128) ; cols (l, o)
        w01 = wsel[:256].reshape(2, 128, GROUPS, GL * CO).transpose(2, 0, 1, 3)
        win01 = winsel[:256].reshape(2, 128, GROUPS, GL * B).transpose(2, 0, 1, 3)
        w2 = wsel[256:].reshape(32, GROUPS, GL * CO).transpose(1, 0, 2)
        win2 = winsel[256:].reshape(32, GROUPS, GL * B).transpose(1, 0, 2)
        in_maps.append({
            "w01": np.ascontiguousarray(w01.reshape(GROUPS * 2 * 128, GL * CO)),
            "win01": np.ascontiguousarray(win01.reshape(GROUPS * 2 * 128, GL * B)),
            "w2": np.ascontiguousarray(w2.reshape(GROUPS * 32, GL * CO)),
            "win2": np.ascontiguousarray(win2.reshape(GROUPS * 32, GL * B)),
        })
    return in_maps


def _assemble_v5(results):
    out = np.empty((B, CO, HO, WO), np.float32)
    qs = np.arange(WO)
    for c in range(N_CORES):
        nreal = RPC if c < N_CORES - 1 else HO - 4 * (N_CORES - 1)
        buf = np.asarray(results[c]["out"])      # [GROUPS*128, 256]
        b5 = buf.reshape(GROUPS, 4, 32, 8, CO)   # (g, strip, 32row, blk, o)
        res = b5[:, qs % 4, :B, qs // 4, :]      # (g?, ...) advanced idx
        # advanced indices qs%4 (dim1) and qs//4 (dim3) -> (31, GROUPS, B, CO)
        out[:, :, 4 * c:4 * c + nreal, :] = res.transpose(2, 3, 1, 0)[:, :, :nreal, :]
    return out


V89_BLK = 4
V89_NBLK = GLP // V89_BLK            # 8 blocks of 4 locs per group
V89_GW = NCHUNK * GLP * CO           # 3072 weight cols per group
V89_GWIN = NCHUNK * GLP * B          # 768 win cols per group
V89_BOUT = V89_BLK * CO              # 128 out cols per block
V89_OROWS = B * V89_BLK              # 32 out rows
V89_OUTC = GROUPS * V89_NBLK * V89_BOUT  # 4096


def _build_nc_v89(repeat=1, three_term=False, dt=None):
    """16-bit blocked kernel, minimal DMA count, split across both HWDGE
    rings. three_term=True computes w≈wh+wl, win≈vh+vl and accumulates
    vh·wh + vh·wl + vl·wh (16-bit products are exact in fp32 -> ~1e-5 rel err).
    """
    if dt is None:
        dt = mybir.dt.bfloat16
    W = GROUPS * V89_GW
    WIN = GROUPS * V89_GWIN
    nc = bacc.Bacc("TRN2", target_bir_lowering=False)
    wh_d = nc.dram_tensor("wh", [CK, W], dt, kind="ExternalInput")
    winh_d = nc.dram_tensor("winh", [CK, WIN], dt, kind="ExternalInput")
    if three_term:
        wl_d = nc.dram_tensor("wl", [CK, W], dt, kind="ExternalInput")
        winl_d = nc.dram_tensor("winl", [CK, WIN], dt, kind="ExternalInput")
    out = nc.dram_tensor("out", [V89_OROWS, V89_OUTC], mybir.dt.float32, kind="ExternalOutput")

    half = W // 2  # 2 groups per ring half
    with tile.TileContext(nc) as tc:
        with (
            tc.tile_pool(name="wp", bufs=2) as wp,
            tc.tile_pool(name="winp", bufs=2) as winp,
            tc.tile_pool(name="pp", bufs=4, space="PSUM") as pp,
            tc.tile_pool(name="op", bufs=2) as op,
        ):
            def body():
                # weight: groups 0-1 via SP ring, groups 2-3 via ACT ring,
                # one piece per group -> compute starts after 1/4 of bytes
                wh = wp.tile([CK, W], dt, tag="wh", name="wh")
                for g in range(2):
                    nc.sync.dma_start(
                        wh[:, g * V89_GW:(g + 1) * V89_GW],
                        wh_d.ap()[:, g * V89_GW:(g + 1) * V89_GW])
                for g in range(2, 4):
                    nc.scalar.dma_start(
                        wh[:, g * V89_GW:(g + 1) * V89_GW],
                        wh_d.ap()[:, g * V89_GW:(g + 1) * V89_GW])
                winh = winp.tile([CK, WIN], dt, tag="winh", name="winh")
                nc.sync.dma_start(winh[:, :WIN // 2], winh_d.ap()[:, :WIN // 2])
                nc.scalar.dma_start(winh[:, WIN // 2:], winh_d.ap()[:, WIN // 2:])
                if three_term:
                    wl = wp.tile([CK, W], dt, tag="wl", name="wl")
                    for g in range(2):
                        nc.scalar.dma_start(
                            wl[:, g * V89_GW:(g + 1) * V89_GW],
                            wl_d.ap()[:, g * V89_GW:(g + 1) * V89_GW])
                    for g in range(2, 4):
                        nc.sync.dma_start(
                            wl[:, g * V89_GW:(g + 1) * V89_GW],
                            wl_d.ap()[:, g * V89_GW:(g + 1) * V89_GW])
                    winl = winp.tile([CK, WIN], dt, tag="winl", name="winl")
                    nc.scalar.dma_start(winl[:, :WIN // 2], winl_d.ap()[:, :WIN // 2])
                    nc.sync.dma_start(winl[:, WIN // 2:], winl_d.ap()[:, WIN // 2:])

                ot = op.tile([V89_OROWS, V89_OUTC], mybir.dt.float32, tag="ot", name="ot")
                for g in range(GROUPS):
                    for bl in range(V89_NBLK):
                        ps = pp.tile([V89_OROWS, V89_BOUT], mybir.dt.float32, tag="ps", name="ps")
                        first = True
                        for c in range(NCHUNK):
                            lo = g * V89_GWIN + c * (GLP * B) + bl * (V89_BLK * B)
                            ro = g * V89_GW + c * (GLP * CO) + bl * V89_BOUT
                            lhs_h = winh[:, lo:lo + V89_BLK * B]
                            rhs_h = wh[:, ro:ro + V89_BOUT]
                            terms = [(lhs_h, rhs_h)]
                            if three_term:
                                terms.append((lhs_h, wl[:, ro:ro + V89_BOUT]))
                                terms.append((winl[:, lo:lo + V89_BLK * B], rhs_h))
                            for ti, (lh, rh) in enumerate(terms):
                                last = (c == NCHUNK - 1) and (ti == len(terms) - 1)
                                nc.tensor.matmul(
                                    ps[:], lhsT=lh, rhs=rh,
                                    start=first, stop=last)
                                first = False
                        nc.vector.tensor_copy(
                            ot[:, (g * V89_NBLK + bl) * V89_BOUT:(g * V89_NBLK + bl + 1) * V89_BOUT],
                            ps[:])
                nc.gpsimd.dma_start(out.ap()[:, :], ot[:])

            if repeat == 1:
                body()
            else:
                with tc.For_i(0, repeat, 1):
                    body()
    nc.compile()
    return nc


def _host_prep_v89(x, weight, three_term=False, npdt=None):
    if npdt is None:
        npdt = ml_dtypes.bfloat16
    x = np.ascontiguousarray(np.asarray(x, dtype=np.float32))
    weight = np.ascontiguousarray(np.asarray(weight, dtype=np.float32))
    wins = np.stack(
        [x[:, :, kh:kh + DH * HO:DH, kw:kw + DW * WO:DW]
         for kh in range(KH) for kw in range(KW)],
        axis=-1,
    )
    W2 = weight[0].transpose(1, 4, 2, 3, 0).reshape(IK, HO, WO, CO)
    W3 = wins.transpose(1, 4, 2, 3, 0).reshape(IK, HO, WO, B)
    qpad = list(range(WO)) + [WO - 1]
    in_maps = []
    for c in range(N_CORES):
        rows = _ROWS_PADDED[c]
        wsel = W2[:, rows][:, :, qpad, :]       # (288, 4, 32, CO)
        winsel = W3[:, rows][:, :, qpad, :]     # (288, 4, 32, B)
        # -> [CK, (group, chunk, locp, {o|b})]
        wstk = np.stack([wsel[CK * cc:CK * (cc + 1)] for cc in range(NCHUNK)], axis=2)
        winstk = np.stack([winsel[CK * cc:CK * (cc + 1)] for cc in range(NCHUNK)], axis=2)
        # (CK, 4, chunk, 32, X) -> (CK, group*chunk*locp*X)
        wfull = wstk.reshape(CK, GROUPS * NCHUNK * GLP * CO)
        winfull = winstk.reshape(CK, GROUPS * NCHUNK * GLP * B)
        m = {}
        wh = wfull.astype(npdt)
        vh = winfull.astype(npdt)
        m["wh"] = np.ascontiguousarray(wh)
        m["winh"] = np.ascontiguousarray(vh)
        if three_term:
            m["wl"] = np.ascontiguousarray(
                (wfull - wh.astype(np.float32)).astype(npdt))
            m["winl"] = np.ascontiguousarray(
                (winfull - vh.astype(np.float32)).astype(npdt))
        in_maps.append(m)
    return in_maps


def _assemble_v89(results):
    BLK = V89_BLK
    NBLK = V89_NBLK
    out = np.empty((B, CO, HO, WO), np.float32)
    idx = np.arange(BLK)
    for c in range(N_CORES):
        nreal = RPC if c < N_CORES - 1 else HO - 4 * (N_CORES - 1)
        buf = np.asarray(results[c]["out"])          # [32, 4096]
        b6 = buf.reshape(BLK, B, GROUPS, NBLK, BLK, CO)
        d = b6[idx, :, :, :, idx, :]                 # (BLK, B, G, NBLK, CO)
        dd = d.transpose(1, 4, 2, 3, 0).reshape(B, CO, GROUPS, NBLK * BLK)
        out[:, :, 4 * c:4 * c + nreal, :] = dd[:, :, :nreal, :WO]
    return out


V10_GTOT = NCHUNK * GLP * CO + NCHUNK * GLP * B   # 3840 cols/group: weight | windows


def _build_nc_v10(repeat=1, dt=None):
    """Like v8h but weight+windows interleaved per group in ONE DRAM tensor:
    one DMA per group (4 input DMAs total) — each dma_start costs ~1.5us of
    serialized ring time here, so DMA count is the dominant knob."""
    if dt is None:
        dt = mybir.dt.float16
    BLK = V89_BLK
    NBLK = V89_NBLK
    gw = V89_GW
    gtot = V10_GTOT
    bout = V89_BOUT
    orows = V89_OROWS
    nc = bacc.Bacc("TRN2", target_bir_lowering=False)
    wx = nc.dram_tensor("wx", [CK, GROUPS * gtot], dt, kind="ExternalInput")
    out = nc.dram_tensor("out", [orows, V89_OUTC], mybir.dt.float32, kind="ExternalOutput")
    with tile.TileContext(nc) as tc:
        with (
            tc.tile_pool(name="wp", bufs=2) as wp,
            tc.tile_pool(name="pp", bufs=4, space="PSUM") as pp,
            tc.tile_pool(name="op", bufs=2) as op,
        ):
            def body():
                t = wp.tile([CK, GROUPS * gtot], dt, tag="t", name="t")
                for g in range(GROUPS):
                    nc.sync.dma_start(t[:, g * gtot:(g + 1) * gtot],
                                      wx.ap()[:, g * gtot:(g + 1) * gtot])
                ot = op.tile([orows, V89_OUTC], mybir.dt.float32, tag="ot", name="ot")
                gout = NBLK * bout
                for g in range(GROUPS):
                    base = g * gtot
                    for bl in range(NBLK):
                        ps = pp.tile([orows, bout], mybir.dt.float32, tag="ps", name="ps")
                        for c in range(NCHUNK):
                            lo = base + gw + c * (GLP * B) + bl * (BLK * B)
                            ro = base + c * (GLP * CO) + bl * bout
                            nc.tensor.matmul(
                                ps[:],
                                lhsT=t[:, lo:lo + BLK * B],
                                rhs=t[:, ro:ro + bout],
                                start=(c == 0), stop=(c == NCHUNK - 1))
                        nc.vector.tensor_copy(
                            ot[:, (g * NBLK + bl) * bout:(g * NBLK + bl + 1) * bout], ps[:])
                    if g == GROUPS - 2:
                        # first 3/4 of the output leaves while group 3 computes
                        nc.gpsimd.dma_start(out.ap()[:, :3 * gout], ot[:, :3 * gout])
                nc.gpsimd.dma_start(out.ap()[:, 3 * gout:], ot[:, 3 * gout:])
            if repeat == 1:
                body()
            else:
                with tc.For_i(0, repeat, 1):
                    body()
    nc.compile()
    return nc


def _host_prep_v10(x, weight, npdt=None):
    if npdt is None:
        npdt = np.float16
    maps = _host_prep_v89(x, weight, three_term=False, npdt=npdt)
    gw = V89_GW
    gwin = V89_GWIN
    out_maps = []
    for m in maps:
        wh = m["wh"].reshape(CK, GROUPS, gw)
        vh = m["winh"].reshape(CK, GROUPS, gwin)
        wx = np.concatenate([wh, vh], axis=2).reshape(CK, GROUPS * V10_GTOT)
        out_maps.append({"wx": np.ascontiguousarray(wx)})
    return out_maps


def _host_prep_v4(x, weight, variant):
    dt, BLK, npdt = V4_CFG[variant]
    x = np.ascontiguousarray(np.asarray(x, dtype=np.float32))
    weight = np.ascontiguousarray(np.asarray(weight, dtype=np.float32))
    wins = np.stack(
        [x[:, :, kh:kh + DH * HO:DH, kw:kw + DW * WO:DW]
         for kh in range(KH) for kw in range(KW)],
        axis=-1,
    )
    W2 = weight[0].transpose(1, 4, 2, 3, 0).reshape(IK, HO, WO, CO)
    W3 = wins.transpose(1, 4, 2, 3, 0).reshape(IK, HO, WO, B)
    qpad = list(range(WO)) + [WO - 1]          # 31 real + 1 dup -> 32
    in_maps = []
    for c in range(N_CORES):
        rows = _ROWS_PADDED[c]
        # (ik, group, locp, {o|b})
        wsel = W2[:, rows][:, :, qpad, :]       # (288, 4, 32, CO)
        winsel = W3[:, rows][:, :, qpad, :]     # (288, 4, 32, B)
        # -> [group, CK, chunk, locp, {o|b}] -> [GROUPS*CK, chunk*locp*{o|b}]
        wstk = np.stack([wsel[CK * cc:CK * (cc + 1)] for cc in range(NCHUNK)], axis=2)
        winstk = np.stack([winsel[CK * cc:CK * (cc + 1)] for cc in range(NCHUNK)], axis=2)
        # wstk: (CK, 4, chunk, 32, CO) -> (4, CK, chunk, 32, CO)
        wstk = wstk.transpose(1, 0, 2, 3, 4).reshape(GROUPS * CK, NCHUNK * GLP * CO)
        winstk = winstk.transpose(1, 0, 2, 3, 4).reshape(GROUPS * CK, NCHUNK * GLP * B)
        in_maps.append({
            "wT": np.ascontiguousarray(wstk.astype(npdt)),
            "winT": np.ascontiguousarray(winstk.astype(npdt)),
        })
    return in_maps


def _assemble_v4(results, variant):
    dt, BLK, _ = V4_CFG[variant]
    NBLK = GLP // BLK
    out = np.empty((B, CO, HO, WO), np.float32)
    idx = np.arange(BLK)
    for c in range(N_CORES):
        nreal = RPC if c < N_CORES - 1 else HO - 4 * (N_CORES - 1)
        buf = np.asarray(results[c]["out"])
        b6 = buf.reshape(BLK, B, GROUPS, NBLK, BLK, CO)
        d = b6[idx, :, :, :, idx, :]            # (BLK, B, GROUPS, NBLK, CO)
        dd = d.transpose(1, 4, 2, 3, 0).reshape(B, CO, GROUPS, NBLK * BLK)
        out[:, :, 4 * c:4 * c + nreal, :] = dd[:, :, :nreal, :WO]
    return out


def _build_nc(repeat=1, variant="v2"):
    nc = bacc.Bacc("TRN2", target_bir_lowering=False)
    wT = nc.dram_tensor("wT", [CK, W_COLS], mybir.dt.float32, kind="ExternalInput")
    winT = nc.dram_tensor("winT", [CK, WIN_COLS], mybir.dt.float32, kind="ExternalInput")
    out_cols = OUT_COLS if variant == "v1" else V2_OUT_COLS
    out_rows = CO if variant == "v1" else 128
    out = nc.dram_tensor("out", [out_rows, out_cols], mybir.dt.float32, kind="ExternalOutput")

    gw = GL * NCHUNK * CO    # weight cols per group
    gwin = GL * NCHUNK * B   # window cols per group
    gout = GL * B            # v1 out cols per group

    with tile.TileContext(nc) as tc:
        with (
            tc.tile_pool(name="wp", bufs=3) as wp,
            tc.tile_pool(name="winp", bufs=3) as winp,
            tc.tile_pool(name="pp", bufs=2, space="PSUM") as pp,
            tc.tile_pool(name="op", bufs=2) as op,
        ):
            def body_v1():
                for g in range(GROUPS):
                    wt = wp.tile([CK, gw], mybir.dt.float32, tag="wt", name="wt")
                    nc.sync.dma_start(wt[:], wT.ap()[:, g * gw:(g + 1) * gw])
                    wint = winp.tile([CK, gwin], mybir.dt.float32, tag="wint", name="wint")
                    nc.sync.dma_start(wint[:], winT.ap()[:, g * gwin:(g + 1) * gwin])

                    ps = pp.tile([CO, gout], mybir.dt.float32, tag="ps", name="ps")
                    for l in range(GL):
                        for c in range(NCHUNK):
                            nc.tensor.matmul(
                                ps[:, l * B:(l + 1) * B],
                                lhsT=wt[:, (l * NCHUNK + c) * CO:(l * NCHUNK + c + 1) * CO],
                                rhs=wint[:, (l * NCHUNK + c) * B:(l * NCHUNK + c + 1) * B],
                                start=(c == 0),
                                stop=(c == NCHUNK - 1),
                            )

                    ot = op.tile([CO, gout], mybir.dt.float32, tag="ot", name="ot")
                    nc.vector.tensor_copy(ot[:], ps[:])
                    nc.sync.dma_start(out.ap()[:, g * gout:(g + 1) * gout], ot[:])

            def body_v2():
                # stationary = windows (8 cols, cheap fp32 self-load);
                # moving = weight (N=32); out[b, o] block at partition
                # offset 32*(l%4) via col-tiling -> 4 concurrent MM strips.
                for g in range(GROUPS):
                    wt = wp.tile([CK, gw], mybir.dt.float32, tag="wt", name="wt")
                    nc.sync.dma_start(wt[:], wT.ap()[:, g * gw:(g + 1) * gw])
                    wint = winp.tile([CK, gwin], mybir.dt.float32, tag="wint", name="wint")
                    nc.sync.dma_start(wint[:], winT.ap()[:, g * gwin:(g + 1) * gwin])

                    # one PSUM tile per col strip -> different banks, so the
                    # 4 strips' matmuls aren't serialized by bank tracking
                    pss = [
                        pp.tile([128, V2_GOUT], mybir.dt.float32,
                                tag=f"ps{j}", name=f"ps{j}", bufs=2)
                        for j in range(4)
                    ]
                    for l in range(GL):
                        j = l % 4
                        blk = l // 4
                        for c in range(NCHUNK):
                            nc.tensor.matmul(
                                pss[j][32 * j:32 * j + B, blk * CO:(blk + 1) * CO],
                                lhsT=wint[:, (l * NCHUNK + c) * B:(l * NCHUNK + c + 1) * B],
                                rhs=wt[:, (l * NCHUNK + c) * CO:(l * NCHUNK + c + 1) * CO],
                                start=(c == 0),
                                stop=(c == NCHUNK - 1),
                                tile_position=(0, 32 * j),
                            )

                    ot = op.tile([128, V2_GOUT], mybir.dt.float32, tag="ot", name="ot")
                    for j in range(4):
                        nc.vector.tensor_copy(
                            ot[32 * j:32 * (j + 1), :],
                            pss[j][32 * j:32 * (j + 1), :],
                        )
                    nc.sync.dma_start(out.ap()[:, g * V2_GOUT:(g + 1) * V2_GOUT], ot[:])

            body = body_v1 if variant == "v1" else body_v2
            if repeat == 1:
                body()
            else:
                with tc.For_i(0, repeat, 1):
                    body()
    nc.compile()
    return nc


def _host_prep(x, weight):
    """Build per-core DMA-ready layouts. Pure indexing/transpose, no math."""
    x = np.ascontiguousarray(np.asarray(x, dtype=np.float32))
    weight = np.ascontiguousarray(np.asarray(weight, dtype=np.float32))

    # windows[b, i, p, q, k] with k = kh*3+kw (matches torch unfold flatten)
    wins = np.stack(
        [x[:, :, kh:kh + DH * HO:DH, kw:kw + DW * WO:DW]
         for kh in range(KH) for kw in range(KW)],
        axis=-1,
    )  # (B, CI, HO, WO, 9)

    # (ik, p, q, o) and (ik, p, q, b)
    W2 = weight[0].transpose(1, 4, 2, 3, 0).reshape(IK, HO, WO, CO)
    W3 = wins.transpose(1, 4, 2, 3, 0).reshape(IK, HO, WO, B)

    in_maps = []
    for c in range(N_CORES):
        rows = _ROWS_PADDED[c]
        wsel = W2[:, rows].reshape(IK, L, CO)
        winsel = W3[:, rows].reshape(IK, L, B)
        # [CK, loc, chunk, {o|b}] — partition r of chunk-c col region holds ik=96c+r
        wT = np.stack([wsel[CK * cc:CK * (cc + 1)] for cc in range(NCHUNK)], axis=2)
        winT = np.stack([winsel[CK * cc:CK * (cc + 1)] for cc in range(NCHUNK)], axis=2)
        in_maps.append({
            "wT": np.ascontiguousarray(wT.reshape(CK, W_COLS)),
            "winT": np.ascontiguousarray(winT.reshape(CK, WIN_COLS)),
        })
    return in_maps


def _assemble(results, variant="v2"):
    out = np.empty((B, CO, HO, WO), np.float32)
    qs = np.arange(WO)
    for c in range(N_CORES):
        nreal = RPC if c < N_CORES - 1 else HO - 4 * (N_CORES - 1)
        buf = np.asarray(results[c]["out"])
        if variant == "v1":
            rr = buf.reshape(CO, RPC, WO, B)
            for j in range(nreal):
                out[:, :, 4 * c + j, :] = rr[:, j, :, :].transpose(2, 0, 1)
        else:
            # buf [128, GROUPS*256]: row = 32*(q%4)+b, col = g*256+(q//4)*32+o
            b4 = buf.reshape(4, 32, GROUPS, 8, CO)
            res = b4[qs % 4, :B, :, qs // 4, :]      # (31, b, g, o)
            out[:, :, 4 * c:4 * c + nreal, :] = res.transpose(1, 3, 2, 0)[:, :, :nreal, :]
    return out


VARIANT = os.environ.get("LC2D_VARIANT", "v10")


def kernel(x, weight, _trace=False, _trace_cores=None):
    if VARIANT == "v10":
        in_maps = _host_prep_v10(x, weight)
    elif VARIANT in ("v8", "v9", "v8h", "v9h"):
        in_maps = _host_prep_v89(
            x, weight, three_term=(VARIANT in ("v9", "v9h")),
            npdt=(np.float16 if VARIANT.endswith("h") else ml_dtypes.bfloat16))
    elif VARIANT in V4_CFG:
        in_maps = _host_prep_v4(x, weight, VARIANT)
    elif VARIANT == "v5":
        in_maps = _host_prep_v5(x, weight)
    else:
        in_maps = _host_prep(x, weight)
    if "nc" not in _NC_CACHE:
        if VARIANT == "v10":
            _NC_CACHE["nc"] = _build_nc_v10(1)
        elif VARIANT in ("v8", "v9", "v8h", "v9h"):
            _NC_CACHE["nc"] = _build_nc_v89(
                1, three_term=(VARIANT in ("v9", "v9h")),
                dt=(mybir.dt.float16 if VARIANT.endswith("h") else mybir.dt.bfloat16))
        elif VARIANT in V4_CFG:
            _NC_CACHE["nc"] = _build_nc_v4(1, VARIANT)
        elif VARIANT == "v5":
            _NC_CACHE["nc"] = _build_nc_v5()
        else:
            _NC_CACHE["nc"] = _build_nc(variant=VARIANT)
    nc = _NC_CACHE["nc"]
    res = run_bass_kernel_spmd(
        nc, in_maps, core_ids=list(range(N_CORES)),
        trace=_trace, trace_cores=_trace_cores,
    )
    if VARIANT in ("v8", "v9", "v8h", "v9h", "v10"):
        out = _assemble_v89(res.results)
    elif VARIANT in V4_CFG:
        out = _assemble_v4(res.results, VARIANT)
    elif VARIANT == "v5":
        out = _assemble_v5(res.results)
    else:
        out = _assemble(res.results, variant=VARIANT)
    if _trace:
        return out, res
    return out


if __name__ == "__main__":
    # quick self-check with random data against a numpy oracle
    rng = np.random.default_rng(0)
    x = rng.standard_normal((B, CI, H, W), dtype=np.float32)
    weight = rng.standard_normal((1, CO, CI, HO, WO, KH * KW), dtype=np.float32)
    wins = np.stack(
        [x[:, :, kh:kh + DH * HO:DH, kw:kw + DW * WO:DW]
         for kh in range(KH) for kw in range(KW)], axis=-1)
    expected = np.einsum("bipqk,oipqk->bopq", wins, weight[0], optimize=True)
    actual = kernel(x, weight)
    err = np.abs(actual - expected).max() / np.abs(expected).max()
    print("max out:", np.abs(expected).max(), "rel err:", err)
    tol = 1e-5 if VARIANT in ("v1", "v2", "v5") else (1e-2 if VARIANT in ("v8", "v4b", "v4b8") else 1e-3)
    assert err < tol, (err, tol)
    print("KERNEL OK")



# revision 7
# speedup vs baseline: 3.2048x; 3.2048x over previous
"""LocallyConnected2d Trainium2 kernel (8-core SPMD).

out[b,o,p,q] = sum_{i,kh,kw} x[b, i, 2p+kh, 2q+kw] * weight[0, o, i, p, q, kh*3+kw]

Shipped variant "v10":
- Shard the H' (=31) output-row dim across 8 cores (4 rows/core; core 7
  gets one duplicated padding row so the SPMD program is uniform). This
  splits the dominant traffic — the 35.4MB per-location weight — 8 ways,
  unlike batch sharding which would replicate it on every core.
- Host-side im2col + layout prep (pure data movement, no math): weight
  and windows are laid out per-core as [96 partitions = (i,k)-chunk,
  free = (group: weight-cols | window-cols)] in fp16, interleaved into a
  SINGLE DRAM tensor so each group needs exactly ONE input DMA (4 total;
  each dma_start costs ~1-2us of serialized ring time here, so DMA count
  matters more than layout).
- Per block of 4 locations: one matmul per contraction chunk,
  lhsT = windows [96, 4*8], rhs = weight [96, 4*32] -> out [32, 128]
  accumulated over the 3 chunks in PSUM; only the 4 diagonal [8, 32]
  tiles are useful (extracted host-side; the 4x moving-side waste is
  free because the kernel is DMA-bound, not PE-bound).
- fp16 (not bf16): same bytes and same PE rate, but 11 mantissa bits
  -> ~3e-4 rel err vs the fp32 reference (bf16 would be ~2.3e-3).
  Variant "v9"/"v9h" (hi+lo split, 3 matmuls) reaches ~4e-6 at ~1.6x
  the time; "v2" is exact fp32 at ~39us.
"""

import os
import numpy as np
import ml_dtypes

import concourse.bacc as bacc
import concourse.mybir as mybir
import concourse.tile as tile
from concourse.bass_utils import run_bass_kernel_spmd

# Problem shapes (hardcoded per contract).
B, CI, H, W = 8, 32, 64, 64
CO = 32
KH = KW = 3
DH = DW = 2
HO = WO = 31
N_CORES = 8
RPC = 4                 # padded H'-rows per core
L = RPC * WO            # 124 locations per core
IK = CI * KH * KW       # 288 contraction
NCHUNK = 3
CK = IK // NCHUNK       # 96 partitions per chunk
GROUPS = RPC            # one compute/DMA group per H'-row
GL = L // GROUPS        # 31 locations per group

W_COLS = L * NCHUNK * CO     # 11904
WIN_COLS = L * NCHUNK * B    # 2976
OUT_COLS = L * B             # 992

_ROWS_PADDED = [[min(4 * c + j, HO - 1) for j in range(RPC)] for c in range(N_CORES)]

_NC_CACHE = {}


V2_GOUT = 256               # psum cols per group in v2: 8 col-blocks x 32 (o)
V2_OUT_COLS = V2_GOUT * GROUPS

# v4: blocked matmuls — BLK locations share one matmul (out is a BLK x BLK
# grid of [b, o] tiles; only the diagonal is useful, extracted host-side).
# fp32r needs moving free dim >= 256 for the 1 cycle/row fast path.
GLP = 32                    # padded locs per group (31 real + 1 dup)
V4_CFG = {
    "v4r": (mybir.dt.float32r, 8, np.float32),
    "v4b": (mybir.dt.bfloat16, 4, ml_dtypes.bfloat16),
    "v4b8": (mybir.dt.bfloat16, 8, ml_dtypes.bfloat16),
}


def _build_nc_v4(repeat, variant):
    dt, BLK, _ = V4_CFG[variant]
    NBLK = GLP // BLK
    gw = NCHUNK * GLP * CO   # 3072 weight cols per group
    gwin = NCHUNK * GLP * B  # 768 win cols per group
    bout = BLK * CO          # out cols per block
    orows = B * BLK          # out rows per block
    out_cols = GROUPS * NBLK * bout

    nc = bacc.Bacc("TRN2", target_bir_lowering=False)
    wT = nc.dram_tensor("wT", [GROUPS * CK, gw], dt, kind="ExternalInput")
    winT = nc.dram_tensor("winT", [GROUPS * CK, gwin], dt, kind="ExternalInput")
    out = nc.dram_tensor("out", [orows, out_cols], mybir.dt.float32, kind="ExternalOutput")

    with tile.TileContext(nc) as tc:
        with (
            tc.tile_pool(name="wp", bufs=3) as wp,
            tc.tile_pool(name="winp", bufs=3) as winp,
            tc.tile_pool(name="pp", bufs=4, space="PSUM") as pp,
            tc.tile_pool(name="op", bufs=4) as op,
        ):
            def body():
                for g in range(GROUPS):
                    wt = wp.tile([CK, gw], dt, tag="wt", name="wt")
                    nc.sync.dma_start(wt[:], wT.ap()[g * CK:(g + 1) * CK, :])
                    wint = winp.tile([CK, gwin], dt, tag="wint", name="wint")
                    nc.sync.dma_start(wint[:], winT.ap()[g * CK:(g + 1) * CK, :])

                    for bl in range(NBLK):
                        ps = pp.tile([orows, bout], mybir.dt.float32, tag="ps", name="ps")
                        for c in range(NCHUNK):
                            nc.tensor.matmul(
                                ps[:],
                                lhsT=wint[:, c * (GLP * B) + bl * (BLK * B):
                                          c * (GLP * B) + (bl + 1) * (BLK * B)],
                                rhs=wt[:, c * (GLP * CO) + bl * bout:
                                       c * (GLP * CO) + (bl + 1) * bout],
                                start=(c == 0),
                                stop=(c == NCHUNK - 1),
                            )
                        ot = op.tile([orows, bout], mybir.dt.float32, tag="ot", name="ot")
                        nc.vector.tensor_copy(ot[:], ps[:])
                        nc.sync.dma_start(
                            out.ap()[:, (g * NBLK + bl) * bout:(g * NBLK + bl + 1) * bout],
                            ot[:],
                        )

            if repeat == 1:
                body()
            else:
                with tc.For_i(0, repeat, 1):
                    body()
    nc.compile()
    return nc


def _build_nc_v5(repeat=1):
    """fp32 exact; all DMAs 128-partition; contraction 128+128+32 with the
    32-row remainder of all 4 groups packed into one 128-row tile."""
    gw = GL * CO     # 992 weight cols per (group, chunk)
    gwin = GL * B    # 248 win cols per (group, chunk)
    nc = bacc.Bacc("TRN2", target_bir_lowering=False)
    w01 = nc.dram_tensor("w01", [GROUPS * 2 * 128, gw], mybir.dt.float32, kind="ExternalInput")
    win01 = nc.dram_tensor("win01", [GROUPS * 2 * 128, gwin], mybir.dt.float32, kind="ExternalInput")
    w2 = nc.dram_tensor("w2", [GROUPS * 32, gw], mybir.dt.float32, kind="ExternalInput")
    win2 = nc.dram_tensor("win2", [GROUPS * 32, gwin], mybir.dt.float32, kind="ExternalInput")
    out = nc.dram_tensor("out", [GROUPS * 128, V2_GOUT], mybir.dt.float32, kind="ExternalOutput")

    with tile.TileContext(nc) as tc:
        with (
            tc.tile_pool(name="wp", bufs=3) as wp,
            tc.tile_pool(name="winp", bufs=3) as winp,
            tc.tile_pool(name="pp", bufs=2, space="PSUM") as pp,
            tc.tile_pool(name="op", bufs=2) as op,
        ):
            def body():
                for g in range(GROUPS):
                    wts, wints = [], []
                    for cc in range(2):
                        wt = wp.tile([128, gw], mybir.dt.float32, tag=f"wt{cc}", name=f"wt{cc}")
                        nc.sync.dma_start(
                            wt[:], w01.ap()[(g * 2 + cc) * 128:(g * 2 + cc + 1) * 128, :])
                        wint = winp.tile([128, gwin], mybir.dt.float32, tag=f"wint{cc}", name=f"wint{cc}")
                        nc.sync.dma_start(
                            wint[:], win01.ap()[(g * 2 + cc) * 128:(g * 2 + cc + 1) * 128, :])
                        wts.append(wt)
                        wints.append(wint)
                    w2t = wp.tile([32, gw], mybir.dt.float32, tag="w2t", name="w2t")
                    nc.sync.dma_start(w2t[:], w2.ap()[g * 32:(g + 1) * 32, :])
                    win2t = winp.tile([32, gwin], mybir.dt.float32, tag="win2t", name="win2t")
                    nc.sync.dma_start(win2t[:], win2.ap()[g * 32:(g + 1) * 32, :])

                    pss = [
                        pp.tile([128, V2_GOUT], mybir.dt.float32,
                                tag=f"ps{j}", name=f"ps{j}", bufs=2)
                        for j in range(4)
                    ]
                    for l in range(GL):
                        j = l % 4
                        blk = l // 4
                        dst = pss[j][32 * j:32 * j + B, blk * CO:(blk + 1) * CO]
                        for cc in range(2):
                            nc.tensor.matmul(
                                dst,
                                lhsT=wints[cc][:, l * B:(l + 1) * B],
                                rhs=wts[cc][:, l * CO:(l + 1) * CO],
                                start=(cc == 0),
                                stop=False,
                                tile_position=(0, 32 * j),
                            )
                        nc.tensor.matmul(
                            dst,
                            lhsT=win2t[:, l * B:(l + 1) * B],
                            rhs=w2t[:, l * CO:(l + 1) * CO],
                            start=False,
                            stop=True,
                            tile_position=(0, 32 * j),
                        )

                    ot = op.tile([128, V2_GOUT], mybir.dt.float32, tag="ot", name="ot")
                    for j in range(4):
                        nc.vector.tensor_copy(
                            ot[32 * j:32 * (j + 1), :],
                            pss[j][32 * j:32 * (j + 1), :],
                        )
                    nc.sync.dma_start(out.ap()[g * 128:(g + 1) * 128, :], ot[:])

            if repeat == 1:
                body()
            else:
                with tc.For_i(0, repeat, 1):
                    body()
    nc.compile()
    return nc


def _host_prep_v5(x, weight):
    x = np.ascontiguousarray(np.asarray(x, dtype=np.float32))
    weight = np.ascontiguousarray(np.asarray(weight, dtype=np.float32))
    wins = np.stack(
        [x[:, :, kh:kh + DH * HO:DH, kw:kw + DW * WO:DW]
         for kh in range(KH) for kw in range(KW)],
        axis=-1,
    )
    W2 = weight[0].transpose(1, 4, 2, 3, 0).reshape(IK, HO, WO, CO)
    W3 = wins.transpose(1, 4, 2, 3, 0).reshape(IK, HO, WO, B)
    in_maps = []
    for c in range(N_CORES):
        rows = _ROWS_PADDED[c]
        wsel = W2[:, rows]       # (288, 4, 31, CO)
        winsel = W3[:, rows]     # (288, 4, 31, B)
        # w01 rows: (g, c01, 128) ; cols (l, o)
        w01 = wsel[:256].reshape(2, 128, GROUPS, GL * CO).transpose(2, 0, 1, 3)
        win01 = winsel[:256].reshape(2, 128, GROUPS, GL * B).transpose(2, 0, 1, 3)
        w2 = wsel[256:].reshape(32, GROUPS, GL * CO).transpose(1, 0, 2)
        win2 = winsel[256:].reshape(32, GROUPS, GL * B).transpose(1, 0, 2)
        in_maps.append({
            "w01": np.ascontiguousarray(w01.reshape(GROUPS * 2 * 128, GL * CO)),
            "win01": np.ascontiguousarray(win01.reshape(GROUPS * 2 * 128, GL * B)),
            "w2": np.ascontiguousarray(w2.reshape(GROUPS * 32, GL * CO)),
            "win2": np.ascontiguousarray(win2.reshape(GROUPS * 32, GL * B)),
        })
    return in_maps


def _assemble_v5(results):
    out = np.empty((B, CO, HO, WO), np.float32)
    qs = np.arange(WO)
    for c in range(N_CORES):
        nreal = RPC if c < N_CORES - 1 else HO - 4 * (N_CORES - 1)
        buf = np.asarray(results[c]["out"])      # [GROUPS*128, 256]
        b5 = buf.reshape(GROUPS, 4, 32, 8, CO)   # (g, strip, 32row, blk, o)
        res = b5[:, qs % 4, :B, qs // 4, :]      # (g?, ...) advanced idx
        # advanced indices qs%4 (dim1) and qs//4 (dim3) -> (31, GROUPS, B, CO)
        out[:, :, 4 * c:4 * c + nreal, :] = res.transpose(2, 3, 1, 0)[:, :, :nreal, :]
    return out


V89_BLK = 4
V89_NBLK = GLP // V89_BLK            # 8 blocks of 4 locs per group
V89_GW = NCHUNK * GLP * CO           # 3072 weight cols per group
V89_GWIN = NCHUNK * GLP * B          # 768 win cols per group
V89_BOUT = V89_BLK * CO              # 128 out cols per block
V89_OROWS = B * V89_BLK              # 32 out rows
V89_OUTC = GROUPS * V89_NBLK * V89_BOUT  # 4096


def _build_nc_v89(repeat=1, three_term=False, dt=None):
    """16-bit blocked kernel, minimal DMA count, split across both HWDGE
    rings. three_term=True computes w≈wh+wl, win≈vh+vl and accumulates
    vh·wh + vh·wl + vl·wh (16-bit products are exact in fp32 -> ~1e-5 rel err).
    """
    if dt is None:
        dt = mybir.dt.bfloat16
    W = GROUPS * V89_GW
    WIN = GROUPS * V89_GWIN
    nc = bacc.Bacc("TRN2", target_bir_lowering=False)
    wh_d = nc.dram_tensor("wh", [CK, W], dt, kind="ExternalInput")
    winh_d = nc.dram_tensor("winh", [CK, WIN], dt, kind="ExternalInput")
    if three_term:
        wl_d = nc.dram_tensor("wl", [CK, W], dt, kind="ExternalInput")
        winl_d = nc.dram_tensor("winl", [CK, WIN], dt, kind="ExternalInput")
    out = nc.dram_tensor("out", [V89_OROWS, V89_OUTC], mybir.dt.float32, kind="ExternalOutput")

    half = W // 2  # 2 groups per ring half
    with tile.TileContext(nc) as tc:
        with (
            tc.tile_pool(name="wp", bufs=2) as wp,
            tc.tile_pool(name="winp", bufs=2) as winp,
            tc.tile_pool(name="pp", bufs=4, space="PSUM") as pp,
            tc.tile_pool(name="op", bufs=2) as op,
        ):
            def body():
                # weight: groups 0-1 via SP ring, groups 2-3 via ACT ring,
                # one piece per group -> compute starts after 1/4 of bytes
                wh = wp.tile([CK, W], dt, tag="wh", name="wh")
                for g in range(2):
                    nc.sync.dma_start(
                        wh[:, g * V89_GW:(g + 1) * V89_GW],
                        wh_d.ap()[:, g * V89_GW:(g + 1) * V89_GW])
                for g in range(2, 4):
                    nc.scalar.dma_start(
                        wh[:, g * V89_GW:(g + 1) * V89_GW],
                        wh_d.ap()[:, g * V89_GW:(g + 1) * V89_GW])
                winh = winp.tile([CK, WIN], dt, tag="winh", name="winh")
                nc.sync.dma_start(winh[:, :WIN // 2], winh_d.ap()[:, :WIN // 2])
                nc.scalar.dma_start(winh[:, WIN // 2:], winh_d.ap()[:, WIN // 2:])
                if three_term:
                    wl = wp.tile([CK, W], dt, tag="wl", name="wl")
                    for g in range(2):
                        nc.scalar.dma_start(
                            wl[:, g * V89_GW:(g + 1) * V89_GW],
                            wl_d.ap()[:, g * V89_GW:(g + 1) * V89_GW])
                    for g in range(2, 4):
                        nc.sync.dma_start(
                            wl[:, g * V89_GW:(g + 1) * V89_GW],
                            wl_d.ap()[:, g * V89_GW:(g + 1) * V89_GW])
                    winl = winp.tile([CK, WIN], dt, tag="winl", name="winl")
                    nc.scalar.dma_start(winl[:, :WIN // 2], winl_d.ap()[:, :WIN // 2])
                    nc.sync.dma_start(winl[:, WIN // 2:], winl_d.ap()[:, WIN // 2:])

                ot = op.tile([V89_OROWS, V89_OUTC], mybir.dt.float32, tag="ot", name="ot")
                for g in range(GROUPS):
                    for bl in range(V89_NBLK):
                        ps = pp.tile([V89_OROWS, V89_BOUT], mybir.dt.float32, tag="ps", name="ps")
                        first = True
                        for c in range(NCHUNK):
                            lo = g * V89_GWIN + c * (GLP * B) + bl * (V89_BLK * B)
                            ro = g * V89_GW + c * (GLP * CO) + bl * V89_BOUT
                            lhs_h = winh[:, lo:lo + V89_BLK * B]
                            rhs_h = wh[:, ro:ro + V89_BOUT]
                            terms = [(lhs_h, rhs_h)]
                            if three_term:
                                terms.append((lhs_h, wl[:, ro:ro + V89_BOUT]))
                                terms.append((winl[:, lo:lo + V89_BLK * B], rhs_h))
                            for ti, (lh, rh) in enumerate(terms):
                                last = (c == NCHUNK - 1) and (ti == len(terms) - 1)
                                nc.tensor.matmul(
                                    ps[:], lhsT=lh, rhs=rh,
                                    start=first, stop=last)
                                first = False
                        nc.vector.tensor_copy(
                            ot[:, (g * V89_NBLK + bl) * V89_BOUT:(g * V89_NBLK + bl + 1) * V89_BOUT],
                            ps[:])
                nc.gpsimd.dma_start(out.ap()[:, :], ot[:])

            if repeat == 1:
                body()
            else:
                with tc.For_i(0, repeat, 1):
                    body()
    nc.compile()
    return nc


def _host_prep_v89(x, weight, three_term=False, npdt=None):
    if npdt is None:
        npdt = ml_dtypes.bfloat16
    x = np.ascontiguousarray(np.asarray(x, dtype=np.float32))
    weight = np.ascontiguousarray(np.asarray(weight, dtype=np.float32))
    wins = np.stack(
        [x[:, :, kh:kh + DH * HO:DH, kw:kw + DW * WO:DW]
         for kh in range(KH) for kw in range(KW)],
        axis=-1,
    )
    W2 = weight[0].transpose(1, 4, 2, 3, 0).reshape(IK, HO, WO, CO)
    W3 = wins.transpose(1, 4, 2, 3, 0).reshape(IK, HO, WO, B)
    qpad = list(range(WO)) + [WO - 1]
    in_maps = []
    for c in range(N_CORES):
        rows = _ROWS_PADDED[c]
        wsel = W2[:, rows][:, :, qpad, :]       # (288, 4, 32, CO)
        winsel = W3[:, rows][:, :, qpad, :]     # (288, 4, 32, B)
        # -> [CK, (group, chunk, locp, {o|b})]
        wstk = np.stack([wsel[CK * cc:CK * (cc + 1)] for cc in range(NCHUNK)], axis=2)
        winstk = np.stack([winsel[CK * cc:CK * (cc + 1)] for cc in range(NCHUNK)], axis=2)
        # (CK, 4, chunk, 32, X) -> (CK, group*chunk*locp*X)
        wfull = wstk.reshape(CK, GROUPS * NCHUNK * GLP * CO)
        winfull = winstk.reshape(CK, GROUPS * NCHUNK * GLP * B)
        m = {}
        wh = wfull.astype(npdt)
        vh = winfull.astype(npdt)
        m["wh"] = np.ascontiguousarray(wh)
        m["winh"] = np.ascontiguousarray(vh)
        if three_term:
            m["wl"] = np.ascontiguousarray(
                (wfull - wh.astype(np.float32)).astype(npdt))
            m["winl"] = np.ascontiguousarray(
                (winfull - vh.astype(np.float32)).astype(npdt))
        in_maps.append(m)
    return in_maps


def _assemble_v89(results):
    BLK = V89_BLK
    NBLK = V89_NBLK
    out = np.empty((B, CO, HO, WO), np.float32)
    idx = np.arange(BLK)
    for c in range(N_CORES):
        nreal = RPC if c < N_CORES - 1 else HO - 4 * (N_CORES - 1)
        buf = np.asarray(results[c]["out"])          # [32, 4096]
        b6 = buf.reshape(BLK, B, GROUPS, NBLK, BLK, CO)
        d = b6[idx, :, :, :, idx, :]                 # (BLK, B, G, NBLK, CO)
        dd = d.transpose(1, 4, 2, 3, 0).reshape(B, CO, GROUPS, NBLK * BLK)
        out[:, :, 4 * c:4 * c + nreal, :] = dd[:, :, :nreal, :WO]
    return out


# ---------------------------------------------------------------------------
# v12: 128-partition layout. Contraction 288 = 128 + 128 + 32; c0/c1 live in
# one [128, 10240] tensor so the bulk DMA rides all 128 partitions / 16 SBUF
# ports (the 96-partition v10 layout idles 4 of 16). The 32-row remainder c2
# is a separate [32, 5120] tensor DMAed on the scalar HWDGE ring, concurrent
# with the sync-ring bulk DMA (matmul base-partition must be 0/32/64, so a
# 4-way partition fold of c2 is not expressible).
# Output is fp16 (tolerance is 2e-2; fp16 rounding ~5e-4) halving out bytes.
# Input is ~31KB/partition -> bufs=2 double-buffering fits, so iteration
# i+1's input DMA overlaps iteration i's compute in the steady state.
V12_LOCS = 128              # 4 padded p-rows x 32 padded q
V12_NBLK = 32               # blocks of BLK=4 locs
V12_BLK = 4
V12_W0 = 0                  # [128, 4096] c0 weight, cols (bl, j, o)
V12_V0 = 4096               # [128, 1024] c0 windows, cols (bl, j, b)
V12_W1 = 5120               # [128, 4096] c1 weight
V12_V1 = 9216               # [128, 1024] c1 windows
V12_COLS = 10240
V12_C2COLS = V12_NBLK * 160  # per block: 128 weight cols | 32 window cols
V12_OROWS = V12_BLK * B     # 32
V12_OUTC = V12_NBLK * V12_BLK * CO  # 4096


def _build_nc_v12(repeat=1, n_in_dma=1):
    dt = mybir.dt.float16
    nc = bacc.Bacc("TRN2", target_bir_lowering=False)
    wx = nc.dram_tensor("wx", [128, V12_COLS], dt, kind="ExternalInput")
    c2 = nc.dram_tensor("c2", [32, V12_C2COLS], dt, kind="ExternalInput")
    out = nc.dram_tensor("out", [V12_OROWS, V12_OUTC], dt, kind="ExternalOutput")
    with tile.TileContext(nc) as tc:
        with (
            tc.tile_pool(name="wp", bufs=2) as wp,
            tc.tile_pool(name="cp", bufs=2) as cp,
            tc.tile_pool(name="pp", bufs=4, space="PSUM") as pp,
            tc.tile_pool(name="op", bufs=2) as op,
        ):
            def body():
                t = wp.tile([128, V12_COLS], dt, tag="t", name="t")
                if n_in_dma == 1:
                    nc.sync.dma_start(t[:], wx.ap()[:, :])
                else:
                    step = V12_COLS // n_in_dma
                    for d in range(n_in_dma):
                        nc.sync.dma_start(
                            t[:, d * step:(d + 1) * step],
                            wx.ap()[:, d * step:(d + 1) * step])
                t2 = cp.tile([32, V12_C2COLS], dt, tag="t2", name="t2")
                nc.scalar.dma_start(t2[:], c2.ap()[:, :])
                ot = op.tile([V12_OROWS, V12_OUTC], dt, tag="ot", name="ot")
                for bl in range(V12_NBLK):
                    ps = pp.tile([V12_OROWS, V12_BLK * CO], mybir.dt.float32,
                                 tag="ps", name="ps")
                    nc.tensor.matmul(
                        ps[:],
                        lhsT=t[:, V12_V0 + 32 * bl:V12_V0 + 32 * bl + 32],
                        rhs=t[:, V12_W0 + 128 * bl:V12_W0 + 128 * bl + 128],
                        start=True, stop=False)
                    nc.tensor.matmul(
                        ps[:],
                        lhsT=t[:, V12_V1 + 32 * bl:V12_V1 + 32 * bl + 32],
                        rhs=t[:, V12_W1 + 128 * bl:V12_W1 + 128 * bl + 128],
                        start=False, stop=False)
                    nc.tensor.matmul(
                        ps[:],
                        lhsT=t2[:, 160 * bl + 128:160 * bl + 160],
                        rhs=t2[:, 160 * bl:160 * bl + 128],
                        start=False, stop=True)
                    dst = ot[:, 128 * bl:128 * (bl + 1)]
                    if bl % 2 == 0:
                        nc.vector.tensor_copy(dst, ps[:])
                    else:
                        nc.scalar.copy(dst, ps[:])
                    if bl == V12_NBLK - 8:
                        nc.gpsimd.dma_start(out.ap()[:, :3 * 1024], ot[:, :3 * 1024])
                nc.gpsimd.dma_start(out.ap()[:, 3 * 1024:], ot[:, 3 * 1024:])
            if repeat == 1:
                body()
            else:
                with tc.For_i(0, repeat, 1):
                    body()
    nc.compile()
    return nc


def _host_prep_v12(x, weight):
    x = np.ascontiguousarray(np.asarray(x, dtype=np.float32))
    weight = np.ascontiguousarray(np.asarray(weight, dtype=np.float32))
    wins = np.stack(
        [x[:, :, kh:kh + DH * HO:DH, kw:kw + DW * WO:DW]
         for kh in range(KH) for kw in range(KW)],
        axis=-1,
    )
    W2 = weight[0].transpose(1, 4, 2, 3, 0).reshape(IK, HO, WO, CO)
    W3 = wins.transpose(1, 4, 2, 3, 0).reshape(IK, HO, WO, B)
    qpad = list(range(WO)) + [WO - 1]
    in_maps = []
    for c in range(N_CORES):
        rows = _ROWS_PADDED[c]
        wsel = W2[:, rows][:, :, qpad, :].reshape(IK, V12_LOCS, CO).astype(np.float16)
        vsel = W3[:, rows][:, :, qpad, :].reshape(IK, V12_LOCS, B).astype(np.float16)
        w0 = wsel[:128].reshape(128, V12_LOCS * CO)
        v0 = vsel[:128].reshape(128, V12_LOCS * B)
        w1 = wsel[128:256].reshape(128, V12_LOCS * CO)
        v1 = vsel[128:256].reshape(128, V12_LOCS * B)
        wx = np.concatenate([w0, v0, w1, v1], axis=1)
        # c2 [32, (bl, 128 w-cols | 32 v-cols)]
        w2 = wsel[256:].reshape(32, V12_NBLK, V12_BLK * CO)
        v2 = vsel[256:].reshape(32, V12_NBLK, V12_BLK * B)
        c2 = np.concatenate([w2, v2], axis=2).reshape(32, V12_C2COLS)
        in_maps.append({"wx": np.ascontiguousarray(wx),
                        "c2": np.ascontiguousarray(c2)})
    return in_maps


def _assemble_v12(results):
    out = np.empty((B, CO, HO, WO), np.float32)
    idx = np.arange(V12_BLK)
    for c in range(N_CORES):
        nreal = RPC if c < N_CORES - 1 else HO - 4 * (N_CORES - 1)
        buf = np.asarray(results[c]["out"]).astype(np.float32)   # [32, 4096]
        b6 = buf.reshape(V12_BLK, B, RPC, 8, V12_BLK, CO)
        d = b6[idx, :, :, :, idx, :]                 # (BLK, B, RPC, 8, CO)
        dd = d.transpose(1, 4, 2, 3, 0).reshape(B, CO, RPC, 32)
        out[:, :, 4 * c:4 * c + nreal, :] = dd[:, :, :nreal, :WO]
    return out


V10_GTOT = NCHUNK * GLP * CO + NCHUNK * GLP * B   # 3840 cols/group: weight | windows


def _build_nc_v10(repeat=1, dt=None):
    """Like v8h but weight+windows interleaved per group in ONE DRAM tensor:
    one DMA per group (4 input DMAs total) — each dma_start costs ~1.5us of
    serialized ring time here, so DMA count is the dominant knob."""
    if dt is None:
        dt = mybir.dt.float16
    BLK = V89_BLK
    NBLK = V89_NBLK
    gw = V89_GW
    gtot = V10_GTOT
    bout = V89_BOUT
    orows = V89_OROWS
    nc = bacc.Bacc("TRN2", target_bir_lowering=False)
    wx = nc.dram_tensor("wx", [CK, GROUPS * gtot], dt, kind="ExternalInput")
    out = nc.dram_tensor("out", [orows, V89_OUTC], mybir.dt.float32, kind="ExternalOutput")
    with tile.TileContext(nc) as tc:
        with (
            tc.tile_pool(name="wp", bufs=2) as wp,
            tc.tile_pool(name="pp", bufs=4, space="PSUM") as pp,
            tc.tile_pool(name="op", bufs=2) as op,
        ):
            def body():
                t = wp.tile([CK, GROUPS * gtot], dt, tag="t", name="t")
                for g in range(GROUPS):
                    nc.sync.dma_start(t[:, g * gtot:(g + 1) * gtot],
                                      wx.ap()[:, g * gtot:(g + 1) * gtot])
                ot = op.tile([orows, V89_OUTC], mybir.dt.float32, tag="ot", name="ot")
                gout = NBLK * bout
                for g in range(GROUPS):
                    base = g * gtot
                    for bl in range(NBLK):
                        ps = pp.tile([orows, bout], mybir.dt.float32, tag="ps", name="ps")
                        for c in range(NCHUNK):
                            lo = base + gw + c * (GLP * B) + bl * (BLK * B)
                            ro = base + c * (GLP * CO) + bl * bout
                            nc.tensor.matmul(
                                ps[:],
                                lhsT=t[:, lo:lo + BLK * B],
                                rhs=t[:, ro:ro + bout],
                                start=(c == 0), stop=(c == NCHUNK - 1))
                        nc.vector.tensor_copy(
                            ot[:, (g * NBLK + bl) * bout:(g * NBLK + bl + 1) * bout], ps[:])
                    if g == GROUPS - 2:
                        # first 3/4 of the output leaves while group 3 computes
                        nc.gpsimd.dma_start(out.ap()[:, :3 * gout], ot[:, :3 * gout])
                nc.gpsimd.dma_start(out.ap()[:, 3 * gout:], ot[:, 3 * gout:])
            if repeat == 1:
                body()
            else:
                with tc.For_i(0, repeat, 1):
                    body()
    nc.compile()
    return nc


def _host_prep_v10(x, weight, npdt=None):
    if npdt is None:
        npdt = np.float16
    maps = _host_prep_v89(x, weight, three_term=False, npdt=npdt)
    gw = V89_GW
    gwin = V89_GWIN
    out_maps = []
    for m in maps:
        wh = m["wh"].reshape(CK, GROUPS, gw)
        vh = m["winh"].reshape(CK, GROUPS, gwin)
        wx = np.concatenate([wh, vh], axis=2).reshape(CK, GROUPS * V10_GTOT)
        out_maps.append({"wx": np.ascontiguousarray(wx)})
    return out_maps


def _host_prep_v4(x, weight, variant):
    dt, BLK, npdt = V4_CFG[variant]
    x = np.ascontiguousarray(np.asarray(x, dtype=np.float32))
    weight = np.ascontiguousarray(np.asarray(weight, dtype=np.float32))
    wins = np.stack(
        [x[:, :, kh:kh + DH * HO:DH, kw:kw + DW * WO:DW]
         for kh in range(KH) for kw in range(KW)],
        axis=-1,
    )
    W2 = weight[0].transpose(1, 4, 2, 3, 0).reshape(IK, HO, WO, CO)
    W3 = wins.transpose(1, 4, 2, 3, 0).reshape(IK, HO, WO, B)
    qpad = list(range(WO)) + [WO - 1]          # 31 real + 1 dup -> 32
    in_maps = []
    for c in range(N_CORES):
        rows = _ROWS_PADDED[c]
        # (ik, group, locp, {o|b})
        wsel = W2[:, rows][:, :, qpad, :]       # (288, 4, 32, CO)
        winsel = W3[:, rows][:, :, qpad, :]     # (288, 4, 32, B)
        # -> [group, CK, chunk, locp, {o|b}] -> [GROUPS*CK, chunk*locp*{o|b}]
        wstk = np.stack([wsel[CK * cc:CK * (cc + 1)] for cc in range(NCHUNK)], axis=2)
        winstk = np.stack([winsel[CK * cc:CK * (cc + 1)] for cc in range(NCHUNK)], axis=2)
        # wstk: (CK, 4, chunk, 32, CO) -> (4, CK, chunk, 32, CO)
        wstk = wstk.transpose(1, 0, 2, 3, 4).reshape(GROUPS * CK, NCHUNK * GLP * CO)
        winstk = winstk.transpose(1, 0, 2, 3, 4).reshape(GROUPS * CK, NCHUNK * GLP * B)
        in_maps.append({
            "wT": np.ascontiguousarray(wstk.astype(npdt)),
            "winT": np.ascontiguousarray(winstk.astype(npdt)),
        })
    return in_maps


def _assemble_v4(results, variant):
    dt, BLK, _ = V4_CFG[variant]
    NBLK = GLP // BLK
    out = np.empty((B, CO, HO, WO), np.float32)
    idx = np.arange(BLK)
    for c in range(N_CORES):
        nreal = RPC if c < N_CORES - 1 else HO - 4 * (N_CORES - 1)
        buf = np.asarray(results[c]["out"])
        b6 = buf.reshape(BLK, B, GROUPS, NBLK, BLK, CO)
        d = b6[idx, :, :, :, idx, :]            # (BLK, B, GROUPS, NBLK, CO)
        dd = d.transpose(1, 4, 2, 3, 0).reshape(B, CO, GROUPS, NBLK * BLK)
        out[:, :, 4 * c:4 * c + nreal, :] = dd[:, :, :nreal, :WO]
    return out


def _build_nc(repeat=1, variant="v2"):
    nc = bacc.Bacc("TRN2", target_bir_lowering=False)
    wT = nc.dram_tensor("wT", [CK, W_COLS], mybir.dt.float32, kind="ExternalInput")
    winT = nc.dram_tensor("winT", [CK, WIN_COLS], mybir.dt.float32, kind="ExternalInput")
    out_cols = OUT_COLS if variant == "v1" else V2_OUT_COLS
    out_rows = CO if variant == "v1" else 128
    out = nc.dram_tensor("out", [out_rows, out_cols], mybir.dt.float32, kind="ExternalOutput")

    gw = GL * NCHUNK * CO    # weight cols per group
    gwin = GL * NCHUNK * B   # window cols per group
    gout = GL * B            # v1 out cols per group

    with tile.TileContext(nc) as tc:
        with (
            tc.tile_pool(name="wp", bufs=3) as wp,
            tc.tile_pool(name="winp", bufs=3) as winp,
            tc.tile_pool(name="pp", bufs=2, space="PSUM") as pp,
            tc.tile_pool(name="op", bufs=2) as op,
        ):
            def body_v1():
                for g in range(GROUPS):
                    wt = wp.tile([CK, gw], mybir.dt.float32, tag="wt", name="wt")
                    nc.sync.dma_start(wt[:], wT.ap()[:, g * gw:(g + 1) * gw])
                    wint = winp.tile([CK, gwin], mybir.dt.float32, tag="wint", name="wint")
                    nc.sync.dma_start(wint[:], winT.ap()[:, g * gwin:(g + 1) * gwin])

                    ps = pp.tile([CO, gout], mybir.dt.float32, tag="ps", name="ps")
                    for l in range(GL):
                        for c in range(NCHUNK):
                            nc.tensor.matmul(
                                ps[:, l * B:(l + 1) * B],
                                lhsT=wt[:, (l * NCHUNK + c) * CO:(l * NCHUNK + c + 1) * CO],
                                rhs=wint[:, (l * NCHUNK + c) * B:(l * NCHUNK + c + 1) * B],
                                start=(c == 0),
                                stop=(c == NCHUNK - 1),
                            )

                    ot = op.tile([CO, gout], mybir.dt.float32, tag="ot", name="ot")
                    nc.vector.tensor_copy(ot[:], ps[:])
                    nc.sync.dma_start(out.ap()[:, g * gout:(g + 1) * gout], ot[:])

            def body_v2():
                # stationary = windows (8 cols, cheap fp32 self-load);
                # moving = weight (N=32); out[b, o] block at partition
                # offset 32*(l%4) via col-tiling -> 4 concurrent MM strips.
                for g in range(GROUPS):
                    wt = wp.tile([CK, gw], mybir.dt.float32, tag="wt", name="wt")
                    nc.sync.dma_start(wt[:], wT.ap()[:, g * gw:(g + 1) * gw])
                    wint = winp.tile([CK, gwin], mybir.dt.float32, tag="wint", name="wint")
                    nc.sync.dma_start(wint[:], winT.ap()[:, g * gwin:(g + 1) * gwin])

                    # one PSUM tile per col strip -> different banks, so the
                    # 4 strips' matmuls aren't serialized by bank tracking
                    pss = [
                        pp.tile([128, V2_GOUT], mybir.dt.float32,
                                tag=f"ps{j}", name=f"ps{j}", bufs=2)
                        for j in range(4)
                    ]
                    for l in range(GL):
                        j = l % 4
                        blk = l // 4
                        for c in range(NCHUNK):
                            nc.tensor.matmul(
                                pss[j][32 * j:32 * j + B, blk * CO:(blk + 1) * CO],
                                lhsT=wint[:, (l * NCHUNK + c) * B:(l * NCHUNK + c + 1) * B],
                                rhs=wt[:, (l * NCHUNK + c) * CO:(l * NCHUNK + c + 1) * CO],
                                start=(c == 0),
                                stop=(c == NCHUNK - 1),
                                tile_position=(0, 32 * j),
                            )

                    ot = op.tile([128, V2_GOUT], mybir.dt.float32, tag="ot", name="ot")
                    for j in range(4):
                        nc.vector.tensor_copy(
                            ot[32 * j:32 * (j + 1), :],
                            pss[j][32 * j:32 * (j + 1), :],
                        )
                    nc.sync.dma_start(out.ap()[:, g * V2_GOUT:(g + 1) * V2_GOUT], ot[:])

            body = body_v1 if variant == "v1" else body_v2
            if repeat == 1:
                body()
            else:
                with tc.For_i(0, repeat, 1):
                    body()
    nc.compile()
    return nc


def _host_prep(x, weight):
    """Build per-core DMA-ready layouts. Pure indexing/transpose, no math."""
    x = np.ascontiguousarray(np.asarray(x, dtype=np.float32))
    weight = np.ascontiguousarray(np.asarray(weight, dtype=np.float32))

    # windows[b, i, p, q, k] with k = kh*3+kw (matches torch unfold flatten)
    wins = np.stack(
        [x[:, :, kh:kh + DH * HO:DH, kw:kw + DW * WO:DW]
         for kh in range(KH) for kw in range(KW)],
        axis=-1,
    )  # (B, CI, HO, WO, 9)

    # (ik, p, q, o) and (ik, p, q, b)
    W2 = weight[0].transpose(1, 4, 2, 3, 0).reshape(IK, HO, WO, CO)
    W3 = wins.transpose(1, 4, 2, 3, 0).reshape(IK, HO, WO, B)

    in_maps = []
    for c in range(N_CORES):
        rows = _ROWS_PADDED[c]
        wsel = W2[:, rows].reshape(IK, L, CO)
        winsel = W3[:, rows].reshape(IK, L, B)
        # [CK, loc, chunk, {o|b}] — partition r of chunk-c col region holds ik=96c+r
        wT = np.stack([wsel[CK * cc:CK * (cc + 1)] for cc in range(NCHUNK)], axis=2)
        winT = np.stack([winsel[CK * cc:CK * (cc + 1)] for cc in range(NCHUNK)], axis=2)
        in_maps.append({
            "wT": np.ascontiguousarray(wT.reshape(CK, W_COLS)),
            "winT": np.ascontiguousarray(winT.reshape(CK, WIN_COLS)),
        })
    return in_maps


def _assemble(results, variant="v2"):
    out = np.empty((B, CO, HO, WO), np.float32)
    qs = np.arange(WO)
    for c in range(N_CORES):
        nreal = RPC if c < N_CORES - 1 else HO - 4 * (N_CORES - 1)
        buf = np.asarray(results[c]["out"])
        if variant == "v1":
            rr = buf.reshape(CO, RPC, WO, B)
            for j in range(nreal):
                out[:, :, 4 * c + j, :] = rr[:, j, :, :].transpose(2, 0, 1)
        else:
            # buf [128, GROUPS*256]: row = 32*(q%4)+b, col = g*256+(q//4)*32+o
            b4 = buf.reshape(4, 32, GROUPS, 8, CO)
            res = b4[qs % 4, :B, :, qs // 4, :]      # (31, b, g, o)
            out[:, :, 4 * c:4 * c + nreal, :] = res.transpose(1, 3, 2, 0)[:, :, :nreal, :]
    return out


VARIANT = os.environ.get("LC2D_VARIANT", "v12")


def kernel(x, weight, _trace=False, _trace_cores=None):
    if VARIANT.startswith("v12"):
        in_maps = _host_prep_v12(x, weight)
    elif VARIANT == "v10":
        in_maps = _host_prep_v10(x, weight)
    elif VARIANT in ("v8", "v9", "v8h", "v9h"):
        in_maps = _host_prep_v89(
            x, weight, three_term=(VARIANT in ("v9", "v9h")),
            npdt=(np.float16 if VARIANT.endswith("h") else ml_dtypes.bfloat16))
    elif VARIANT in V4_CFG:
        in_maps = _host_prep_v4(x, weight, VARIANT)
    elif VARIANT == "v5":
        in_maps = _host_prep_v5(x, weight)
    else:
        in_maps = _host_prep(x, weight)
    if "nc" not in _NC_CACHE:
        if VARIANT.startswith("v12"):
            _NC_CACHE["nc"] = _build_nc_v12(
                1, n_in_dma=(4 if VARIANT == "v12q" else 1))
        elif VARIANT == "v10":
            _NC_CACHE["nc"] = _build_nc_v10(1)
        elif VARIANT in ("v8", "v9", "v8h", "v9h"):
            _NC_CACHE["nc"] = _build_nc_v89(
                1, three_term=(VARIANT in ("v9", "v9h")),
                dt=(mybir.dt.float16 if VARIANT.endswith("h") else mybir.dt.bfloat16))
        elif VARIANT in V4_CFG:
            _NC_CACHE["nc"] = _build_nc_v4(1, VARIANT)
        elif VARIANT == "v5":
            _NC_CACHE["nc"] = _build_nc_v5()
        else:
            _NC_CACHE["nc"] = _build_nc(variant=VARIANT)
    nc = _NC_CACHE["nc"]
    res = run_bass_kernel_spmd(
        nc, in_maps, core_ids=list(range(N_CORES)),
        trace=_trace, trace_cores=_trace_cores,
    )
    if VARIANT.startswith("v12"):
        out = _assemble_v12(res.results)
    elif VARIANT in ("v8", "v9", "v8h", "v9h", "v10"):
        out = _assemble_v89(res.results)
    elif VARIANT in V4_CFG:
        out = _assemble_v4(res.results, VARIANT)
    elif VARIANT == "v5":
        out = _assemble_v5(res.results)
    else:
        out = _assemble(res.results, variant=VARIANT)
    if _trace:
        return out, res
    return out


if __name__ == "__main__":
    # quick self-check with random data against a numpy oracle
    rng = np.random.default_rng(0)
    x = rng.standard_normal((B, CI, H, W), dtype=np.float32)
    weight = rng.standard_normal((1, CO, CI, HO, WO, KH * KW), dtype=np.float32)
    wins = np.stack(
        [x[:, :, kh:kh + DH * HO:DH, kw:kw + DW * WO:DW]
         for kh in range(KH) for kw in range(KW)], axis=-1)
    expected = np.einsum("bipqk,oipqk->bopq", wins, weight[0], optimize=True)
    actual = kernel(x, weight)
    err = np.abs(actual - expected).max() / np.abs(expected).max()
    print("max out:", np.abs(expected).max(), "rel err:", err)
    tol = 1e-5 if VARIANT in ("v1", "v2", "v5") else (1e-2 if VARIANT in ("v8", "v4b", "v4b8") else 1e-3)
    assert err < tol, (err, tol)
    print("KERNEL OK")

